# revision 29
# baseline (speedup 1.0000x reference)
"""Trainium2 Bass kernel for the MANN network (LSTM scan + memory-write scan).

Self-contained: hardcodes all shapes. kernel(**inputs) takes full numpy inputs
and returns the full [128, 40] final memory matrix.

Structure (single-core program, replicated on 8 cores via SPMD):
  Phase A (batch): GXT[p, jj, t] = gate pre-activations from x/y (PE matmuls -> DRAM)
  Loop over 32 chunks of 128 steps:
    B(c): 128 sequential LSTM steps (tanh-only nonlinearities, fp16 W_hh stationary)
    C(c): chunk keys/sigma batch matmuls + per-chunk broadcast tiles
    D(c-1): 128 sequential memory-update steps in slot-major [128,40] layout:
            the M write is a fused scalar_tensor_tensor (per-partition ww), the
            score/rownorm reductions are fused stt+accum reductions along
            the free axis, softmax exp uses the real Exp table (same ACT set as
            tanh/sign) with a per-partition rsqrt scale, and the softmax sum
            comes back as a [128,1] broadcast via a ones-matrix matmul.
            The rsqrt Newton state is refreshed every 2nd step and consumed one
            step stale (validated to 9e-5 rel err).
"""

import sys

import numpy as np

# concourse (Bass) lives in the TRN RL repo; make it importable regardless of cwd
for _p in ("/opt/trn_rl_repo", "/root/.axon_site/_ro/trn_rl_repo"):
    try:
        import concourse  # noqa: F401
        break
    except ImportError:
        if _p not in sys.path:
            sys.path.insert(0, _p)

T, D, F, H, NS, KD = 4096, 512, 256, 200, 128, 40
TC = 128                  # steps per chunk
NCH = T // TC             # 32 chunks
G4P = 1024                # padded gate vector (4 gates x 256)
QUAKE_F = 1597463007.0    # 0x5f3759df as float
N_CORES = 8


# ---------------------------------------------------------------- host prep --
def _prep(inputs):
    f32 = np.float32
    x = np.ascontiguousarray(inputs["x_train"], f32)
    y = np.ascontiguousarray(inputs["y_train"], f32)
    W_in = np.asarray(inputs["W_in"], f32)
    b_in = np.asarray(inputs["b_in"], f32)
    W_ih = np.asarray(inputs["W_ih"], f32)
    W_hh = np.asarray(inputs["W_hh"], f32)
    b_ih = np.asarray(inputs["b_ih"], f32)
    b_hh = np.asarray(inputs["b_hh"], f32)
    W_k = np.asarray(inputs["W_k"], f32)
    b_k = np.asarray(inputs["b_k"], f32)
    W_s = np.asarray(inputs["W_s"], f32)
    b_s = np.asarray(inputs["b_s"], f32)

    # Gate reorder (i, f, gg, o) -> (i, f, o, gg); sigmoid gates scaled by 0.5
    # (sigmoid(v) = 0.5*tanh(0.5 v)+0.5), pad each gate 200 -> 256 rows.
    gate_src = [0, 1, 3, 2]
    scale = [0.5, 0.5, 0.5, 1.0]
    b_tot = b_ih + b_hh
    Wtil = np.zeros((G4P, F + 2), f32)   # cols 0:256 = x feats, 256 = y, 257 = bias
    Whhp = np.zeros((G4P, H), f32)
    for g in range(4):
        src = gate_src[g]
        rows = slice(256 * g, 256 * g + H)
        Wtil[rows, 0:F + 1] = scale[g] * W_ih[200 * src:200 * src + H, :]
        Wtil[rows, F + 1] = scale[g] * b_tot[200 * src:200 * src + H]
        Whhp[rows, :] = scale[g] * W_hh[200 * src:200 * src + H, :]

    watil_t = np.ascontiguousarray(Wtil.T)                     # [258, 1024] f32
    # h is carried as 2h (h2 = (tanh_o+1)*tanh(c)); absorb the 0.5 into the
    # weights that consume h: W_hh and the hid rows of W_k/W_s.
    whhT = np.ascontiguousarray(0.5 * Whhp.T).astype(np.float16)   # [200, 1024] fp16
    # keys/sigma weights: rows 0:200 hid, 200:224 zero pad, 224 bias; col 40 scaled W_s
    wks = np.zeros((225, KD + 1), f32)
    wks[0:H, 0:KD] = 0.5 * W_k
    wks[224, 0:KD] = b_k
    wks[0:H, KD] = 0.25 * W_s[:, 0]
    wks[224, KD] = 0.5 * b_s[0]
    wks = wks.astype(np.float16)

    ysh1 = np.zeros((2, T), f32)          # row0 = y_shift, row1 = ones
    ysh1[0, 1:] = y[:-1, 0]
    ysh1[1, :] = 1.0
    return {
        "x_train": x.astype(np.float16),
        "ysh1": ysh1.astype(np.float16),
        "watil_t": watil_t.astype(np.float16),
        "whht": whhT,
        "wks": wks,
        "w_in": np.ascontiguousarray(W_in).astype(np.float16),
        "b_in": np.ascontiguousarray(b_in.reshape(2, 128)),   # [m, p] -> load as [128,2]
    }


# ------------------------------------------------------------- bass program --
def build(nc, tc):
    import concourse.bass as bass
    from concourse import mybir
    from concourse.bass import ds

    f32 = mybir.dt.float32
    f16 = mybir.dt.float16
    u32 = mybir.dt.uint32
    AF = mybir.ActivationFunctionType
    OP = mybir.AluOpType

    x_d = nc.dram_tensor("x_train", [T, D], f16, kind="ExternalInput")
    y_d = nc.dram_tensor("ysh1", [2, T], f16, kind="ExternalInput")
    watil_d = nc.dram_tensor("watil_t", [F + 2, G4P], f16, kind="ExternalInput")
    whht_d = nc.dram_tensor("whht", [H, G4P], f16, kind="ExternalInput")
    wks_d = nc.dram_tensor("wks", [225, KD + 1], f16, kind="ExternalInput")
    win_d = nc.dram_tensor("w_in", [D, F], f16, kind="ExternalInput")
    bin_d = nc.dram_tensor("b_in", [2, 128], f32, kind="ExternalInput")
    m_out = nc.dram_tensor("m_out", [NS, KD], f32, kind="ExternalOutput")
    gxt_d = nc.dram_tensor("gxt", [128, 8, T + TC], f16)  # internal scratch

    from contextlib import ExitStack
    stack = ExitStack()

    singles = stack.enter_context(tc.tile_pool(name="singles", bufs=1))

    # ---------------- persistent loop tiles ----------------
    whh_lo = singles.tile([128, G4P], f16)
    whh_hi = singles.tile([72, G4P], f16)
    wks_lo = singles.tile([128, KD + 1], f16)
    wks_hi = singles.tile([97, KD + 1], f16)
    ident128 = singles.tile([128, 128], f32)
    ones40 = singles.tile([40, 1], f32)
    ones_row16 = singles.tile([1, 128], f16)
    ones_row32 = singles.tile([1, 128], f32)
    ones128sq = singles.tile([128, 128], f32)
    ident16 = singles.tile([128, 128], f16)

    h16 = singles.tile([128, 2], f16)       # 2h (col0 = h[0:128], col1 = h[128:200]+pad)
    tgx = singles.tile([128, 10], f32)      # cols 0:8 tanh(gates); cols 8:10 = 2c state
    gsum = singles.tile([128, 4], f32)      # (tg+1)*[g|c2] scratch
    thc = singles.tile([128, 2], f32)

    # D-scan state, slot-major
    Ms = singles.tile([NS, KD], f32)        # memory [slot, key]
    norm2 = singles.tile([128, 1], f32)
    rn = singles.tile([128, 1], f32)        # rsqrt(row-norm^2), newton state
    rn2 = singles.tile([128, 1], f32)
    nt2 = singles.tile([128, 1], f32)
    nt3 = singles.tile([128, 1], f32)
    qu1 = singles.tile([128, 1], u32)
    qf1 = singles.tile([128, 1], f32)
    qf2 = singles.tile([128, 1], f32)
    qy0 = singles.tile([128, 1], u32)
    e_col = singles.tile([128, 1], f32)
    rs_bc = singles.tile([128, 1], f32)
    t1c = singles.tile([128, 1], f32)
    ww = singles.tile([128, 1], f32)
    scores = singles.tile([128, 1], f32)
    p_col = singles.tile([128, 1], f32)
    scr = singles.tile([128, 2, KD], f32)   # ttr full-output scratch (ping)
    scr2 = singles.tile([128, KD], f32)     # ttr scratch for norm2

    # ping-pong chunk tiles
    gx_tile = [singles.tile([128, 8, TC], f16, tag=f"gx{p}", name=f"gx{p}") for p in range(2)]
    hidc_a = [singles.tile([128, TC], f16, tag=f"ha{p}", name=f"ha{p}") for p in range(2)]
    hidc_b = [singles.tile([97, TC], f16, tag=f"hb{p}", name=f"hb{p}") for p in range(2)]
    keysc = [singles.tile([128, KD + 1], f32, tag=f"kc{p}", name=f"kc{p}") for p in range(2)]
    sigrow = [singles.tile([1, 128], f32, tag=f"sr{p}", name=f"sr{p}") for p in range(2)]
    sig_bc = [singles.tile([128, TC], f32, tag=f"sb{p}", name=f"sb{p}") for p in range(2)]
    omsig_bc = [singles.tile([128, TC], f32, tag=f"ob{p}", name=f"ob{p}") for p in range(2)]
    alpha_bc = [singles.tile([128, TC], f32, tag=f"ab{p}", name=f"ab{p}") for p in range(2)]
    kt16 = [singles.tile([40, 128], f16, tag=f"kt{p}", name=f"kt{p}") for p in range(2)]
    kn16 = [singles.tile([40, 128], f16, tag=f"knt{p}", name=f"knt{p}") for p in range(2)]
    krd16 = [singles.tile([1, KD, TC], f16, tag=f"kr{p}", name=f"kr{p}") for p in range(2)]
    knrd16 = [singles.tile([1, KD, TC], f16, tag=f"knr{p}", name=f"knr{p}") for p in range(2)]
    sigpad = singles.tile([128, 32], f32)
    sigtr = singles.tile([128, 32], f32)
    kabs = singles.tile([40, 128], f32)
    arow_sb = singles.tile([1, 128], f32)

    # ---------------- static init ----------------
    nc.sync.dma_start(whh_lo[:], whht_d[0:128, :])
    nc.sync.dma_start(whh_hi[:], whht_d[128:200, :])
    nc.sync.dma_start(wks_lo[:], wks_d[0:128, :])
    nc.sync.dma_start(wks_hi[:], wks_d[128:225, :])
    nc.vector.memset(ident128[:], 1.0)
    nc.gpsimd.affine_select(ident128[:], ident128[:], [[-1, 128]], OP.is_equal, 0.0,
                            base=0, channel_multiplier=1)
    nc.vector.tensor_copy(ident16[:], ident128[:])
    nc.vector.memset(ones40[:], 1.0)
    nc.vector.memset(ones_row16[:], 1.0)
    nc.vector.memset(ones_row32[:], 1.0)
    nc.vector.memset(ones128sq[:], 1.0)
    nc.vector.memset(h16[:], 0.0)
    nc.vector.memset(tgx[:], 0.0)
    nc.vector.memset(Ms[:], 1e-6)
    nc.vector.memset(rn[:], 1.0)
    nc.vector.memset(e_col[:], 0.0)
    nc.vector.memset(e_col[0:1, :], 1.0)
    nc.vector.memset(sigpad[:], 0.0)
    for p in range(2):
        nc.vector.memset(hidc_b[p][:], 0.0)
        nc.vector.memset(hidc_b[p][96:97, :], 1.0)

    # ---------------- phase A: GXT ----------------
    with tc.tile_pool(name="pha1", bufs=1) as pha1, \
         tc.tile_pool(name="pha", bufs=3) as pha, \
         tc.tile_pool(name="pha_ps", bufs=2, space="PSUM") as pha_ps:
        xT = [pha1.tile([128, T], f16, tag=f"xT{k}", name=f"xT{k}") for k in range(4)]
        for k in range(4):
            nc.sync.dma_start(xT[k][:], x_d[:, 128 * k:128 * (k + 1)].rearrange("t d -> d t"))
        win_sb = pha1.tile([128, 4, F], f16)
        nc.sync.dma_start(win_sb[:], win_d.rearrange("(k p) f -> p k f", p=128))
        binc = pha1.tile([128, 2], f32)
        nc.sync.dma_start(binc[:], bin_d.rearrange("m p -> p m"))
        wat0 = pha1.tile([128, G4P], f16)
        wat1 = pha1.tile([128, G4P], f16)
        wat2 = pha1.tile([2, G4P], f16)
        nc.sync.dma_start(wat0[:], watil_d[0:128, :])
        nc.sync.dma_start(wat1[:], watil_d[128:256, :])
        nc.sync.dma_start(wat2[:], watil_d[256:258, :])

        xys0 = pha1.tile([128, T], f16)
        xys1 = pha1.tile([128, T], f16)
        xys2 = pha1.tile([2, T], f16)
        nc.sync.dma_start(xys2[:], y_d[:])

        # xsT = W_in.T @ x.T  (+ b_in)
        for m in range(2):
            dst = xys0 if m == 0 else xys1
            for n in range(8):
                ps = pha_ps.tile([128, 512], f32, tag="psA")
                for k in range(4):
                    nc.tensor.matmul(ps[:], win_sb[:, k, 128 * m:128 * (m + 1)],
                                     xT[k][:, 512 * n:512 * (n + 1)],
                                     start=(k == 0), stop=(k == 3))
                nc.vector.tensor_scalar(dst[:, 512 * n:512 * (n + 1)], ps[:],
                                        binc[:, m:m + 1], None, OP.add)

        # GXT = Wtil_aug.T-slices @ xysT -> DRAM (n outer: early cols first)
        for n in range(8):
            for jj in range(8):
                ps = pha_ps.tile([128, 512], f32, tag="psA")
                nc.tensor.matmul(ps[:], wat0[:, 128 * jj:128 * (jj + 1)],
                                 xys0[:, 512 * n:512 * (n + 1)], start=True, stop=False)
                nc.tensor.matmul(ps[:], wat1[:, 128 * jj:128 * (jj + 1)],
                                 xys1[:, 512 * n:512 * (n + 1)], start=False, stop=False)
                nc.tensor.matmul(ps[:], wat2[:, 128 * jj:128 * (jj + 1)],
                                 xys2[:, 512 * n:512 * (n + 1)], start=False, stop=True)
                stg = pha.tile([128, 512], f16, tag="stgA")
                nc.vector.tensor_copy(stg[:], ps[:])
                nc.sync.dma_start(gxt_d[:, jj, 512 * n:512 * (n + 1)], stg[:])

    # loop-phase PSUM (allocated after phase A pools close: 8 banks exactly)
    psingles = stack.enter_context(tc.tile_pool(name="psingles", bufs=1, space="PSUM"))
    gP = psingles.tile([128, 8], f32)
    kraw = psingles.tile([128, KD + 1], f32)
    ktr = psingles.tile([40, 128], f32)
    rowps = psingles.tile([1, 128], f32)
    bcps = psingles.tile([128, 128], f32)
    se_bc = psingles.tile([128, 1], f32)
    kbknb = psingles.tile([128, 2, 2, KD], f32)   # [*, ping, {kb,knb}, key]
    mtp = psingles.tile([128, 8], f32)            # spare
    nc.vector.memset(se_bc[:], 1.0)

    # ---------------- chunk emitters ----------------
    def emit_B_step(c, gx, s):
        par = c % 2
        ha, hb = hidc_a[par], hidc_b[par]
        # gates = gx + Whh @ h. The gx term enters PSUM via an identity matmul
        # (depends only on the prefetched gx tile, so it runs ahead of the
        # h-dependent matmuls and removes a DVE add + 2 sync hops per step).
        nc.tensor.matmul(gP[:], ident16[:], gx[:, :, s], start=True, stop=False)
        for kc in range(2):
            slab = whh_lo if kc == 0 else whh_hi
            rhs = h16[:, 0:1] if kc == 0 else h16[0:72, 1:2]
            for jj in range(8):
                nc.tensor.matmul(gP[:, jj:jj + 1], slab[:, 128 * jj:128 * (jj + 1)],
                                 rhs, start=False, stop=(kc == 1 and jj == 7))
        nc.scalar.activation(tgx[:, 0:8], gP[:], AF.Tanh)
        # c carried as c2 = 2c:  gsum = (tg_if + 1) * [g0 g1 c2_0 c2_1]
        #   = [2*sig_i*g | 2*sig_f*c2];  c2' = gsum_i + 0.5*gsum_f
        nc.vector.scalar_tensor_tensor(gsum[:], tgx[:, 0:4], 1.0, tgx[:, 6:10],
                                       OP.add, OP.mult)
        nc.vector.scalar_tensor_tensor(tgx[:, 8:10], gsum[:, 2:4], 0.5, gsum[:, 0:2],
                                       OP.mult, OP.add)
        nc.scalar.activation(thc[:], tgx[:, 8:10], AF.Tanh, scale=0.5)
        # h carried as 2h = (tanh_o + 1) * tanh(c); 0.5 absorbed in W_hh/W_k/W_s
        nc.vector.scalar_tensor_tensor(h16[:], tgx[:, 4:6], 1.0, thc[:],
                                       OP.add, OP.mult)
        nc.gpsimd.tensor_copy(ha[:, s:s + 1], h16[:, 0:1])
        nc.gpsimd.tensor_copy(hb[0:72, s:s + 1], h16[0:72, 1:2])

    def emit_krd_dma(par):
        nc.sync.dma_start(krd16[par][:], kt16[par][:])
        nc.sync.dma_start(knrd16[par][:], kn16[par][:])

    def emit_C(c, do_dma=True):
        par = c % 2
        nc.tensor.matmul(kraw[:], hidc_a[par][:], wks_lo[:], start=True, stop=False)
        nc.tensor.matmul(kraw[:], hidc_b[par][:], wks_hi[:], start=False, stop=True)
        nc.scalar.activation(keysc[par][:], kraw[:], AF.Tanh)
        # sigma row extraction via 32x32 stream-transpose blocks
        nc.vector.tensor_scalar(sigpad[:, 0:1], keysc[par][:, KD:KD + 1], 0.5, 0.5,
                                OP.mult, OP.add)
        nc.vector.transpose(sigtr[:], sigpad[:])
        for i in range(4):
            nc.gpsimd.tensor_copy(sigrow[par][0:1, 32 * i:32 * (i + 1)],
                                  sigtr[32 * i:32 * i + 1, 0:32])
        # keys^T [40,128]; fp16 k / sign(k) row forms via Pool-queue DMAs (off
        # the SP queue so they never delay the gx prefetch)
        nc.tensor.transpose(ktr[:], keysc[par][:, 0:KD], ident128[:])
        nc.vector.tensor_copy(kt16[par][:], ktr[:])
        nc.scalar.activation(kn16[par][:], ktr[:], AF.Sign)
        nc.scalar.activation(kabs[:], ktr[:], AF.Abs)
        if do_dma:
            emit_krd_dma(par)
        # alpha row = col-sums of |k|; broadcast tiles
        nc.tensor.matmul(rowps[:], ones40[:], kabs[:], start=True, stop=True)
        nc.vector.tensor_copy(arow_sb[:], rowps[:])
        nc.tensor.matmul(bcps[:], ones_row32[:], arow_sb[:], start=True, stop=True)
        nc.scalar.activation(alpha_bc[par][:], bcps[:], AF.Copy)
        nc.tensor.matmul(bcps[:], ones_row32[:], sigrow[par][:], start=True, stop=True)
        nc.scalar.activation(sig_bc[par][:], bcps[:], AF.Copy)
        nc.vector.tensor_scalar(omsig_bc[par][:], sig_bc[par][:], -1.0, 1.0,
                                OP.mult, OP.add)

    def emit_D_step(c, s, cold=False):
        par = c % 2
        sp = s % 2
        kb = kbknb[:, sp, 0, :]
        knb = kbknb[:, sp, 1, :]
        # broadcast k / kn rows to all 128 partitions (PE, chunk data only)
        nc.tensor.matmul(kb, ones_row16[:], krd16[par][0:1, :, s], start=True, stop=True)
        nc.tensor.matmul(knb, ones_row16[:], knrd16[par][0:1, :, s], start=True, stop=True)
        # p = rowdot(Ms, knb) with Ms as of step s-1 (runs during prev exp)
        nc.vector.scalar_tensor_tensor(scr[:, sp, :], Ms[:], 1.0, knb,
                                       OP.mult, OP.mult, accum_out=p_col[:])
        # softmax chain: rs -> ww -> scores
        nc.vector.reciprocal(rs_bc[:], se_bc[:])
        nc.vector.scalar_tensor_tensor(t1c[:], rs_bc[:], sig_bc[par][:, s:s + 1],
                                       e_col[:], OP.mult, OP.mult)
        nc.vector.tensor_scalar(ww[:], t1c[:], omsig_bc[par][:, s:s + 1], None, OP.add)
        nc.vector.scalar_tensor_tensor(scores[:], ww[:], alpha_bc[par][:, s:s + 1],
                                       p_col[:], OP.mult, OP.add)
        # memory write: Ms += ww * kb   (per-partition scalar ww)
        nc.vector.scalar_tensor_tensor(Ms[:], kb, ww[:], Ms[:], OP.mult, OP.add)
        if cold:
            # exact row-norm^2 + quake rsqrt + 3 newton iters, fresh rn
            nc.vector.scalar_tensor_tensor(scr2[:], Ms[:], 1.0, Ms[:],
                                           OP.mult, OP.mult, accum_out=norm2[:])
            nc.vector.tensor_scalar(qu1[:], norm2.bitcast(u32)[:], 1, None,
                                    OP.logical_shift_right)
            nc.vector.tensor_copy(qf1[:], qu1[:])
            nc.vector.tensor_scalar(qf2[:], qf1[:], -1.0, QUAKE_F, OP.mult, OP.add)
            nc.vector.tensor_copy(qy0[:], qf2[:])
            nc.vector.tensor_copy(rn[:], qy0.bitcast(f32)[:])
            for _ in range(3):
                nc.vector.tensor_mul(rn2[:], rn[:], rn[:])
                nc.vector.tensor_mul(nt2[:], rn2[:], norm2[:])
                nc.vector.tensor_scalar(nt3[:], nt2[:], -0.5, 1.5, OP.mult, OP.add)
                nc.vector.tensor_mul(rn[:], rn[:], nt3[:])
        # softmax exp with per-partition rsqrt scale; sum broadcast via PE
        nc.scalar.activation(e_col[:], scores[:], AF.Exp, scale=rn[:])
        nc.tensor.matmul(se_bc[:], ones128sq[:], e_col[:], start=True, stop=True)
        if not cold and sp == 0:
            # refresh rn (consumed one step stale; emitted after the exp above)
            nc.vector.scalar_tensor_tensor(scr2[:], Ms[:], 1.0, Ms[:],
                                           OP.mult, OP.mult, accum_out=norm2[:])
            nc.vector.tensor_mul(rn2[:], rn[:], rn[:])
            nc.vector.tensor_mul(nt2[:], rn2[:], norm2[:])
            nc.vector.tensor_scalar(nt3[:], nt2[:], -0.5, 1.5, OP.mult, OP.add)
            nc.vector.tensor_mul(rn[:], rn[:], nt3[:])

    def emit_BD(cB, gx, cD, coldD=False):
        """Memory chunk cD (block) then LSTM chunk cB (block); the tile
        scheduler/FIFOs interleave the two independent chains at runtime."""
        if cD is not None:
            for s in range(TC):
                emit_D_step(cD, s, coldD)
        for s in range(TC):
            emit_B_step(cB, gx, s)

    # ---------------- prologue + loop + epilogue ----------------
    nc.sync.dma_start(gx_tile[0][:], gxt_d[:, :, 0:TC])
    nc.sync.dma_start(gx_tile[1][:], gxt_d[:, :, TC:2 * TC])
    emit_BD(0, gx_tile[0], None)
    emit_C(0)

    # static section for chunk 1 (runs the only cold D chunk).  The odd-parity
    # row-DMAs are deferred to the next body start so they are never in flight
    # at the For_i all-engine barrier (each barrier otherwise stalls ~13us
    # waiting on the two 7.9us transfers issued at the body's end).
    nc.sync.dma_start(gx_tile[0][:], gxt_d[:, :, 2 * TC:3 * TC])
    emit_BD(1, gx_tile[1], 0, coldD=True)
    emit_C(1, do_dma=False)

    def emit_section(c_par, iv_expr, defer_dma=False):
        """Section for B/C chunk with static parity c_par, dynamic index iv_expr."""
        nc.sync.dma_start(gx_tile[(c_par + 1) % 2][:],
                          gxt_d[:, :, ds((iv_expr + 1) * TC, TC)])
        emit_BD(c_par, gx_tile[c_par % 2], c_par - 1)
        emit_C(c_par, do_dma=not defer_dma)

    with tc.For_i(0, 14) as i:
        emit_krd_dma(1)              # chunk 2i+1's rows (tiles persist)
        emit_section(2, 2 * i + 2)
        emit_section(3, 2 * i + 3, defer_dma=True)

    emit_krd_dma(1)                  # chunk 29's rows
    emit_BD(30, gx_tile[0], 29)
    emit_C(30)
    nc.sync.dma_start(gx_tile[1][:], gxt_d[:, :, 31 * TC:32 * TC])
    emit_BD(31, gx_tile[1], 30)
    emit_C(31)
    for s in range(TC):
        emit_D_step(31, s)

    # output: Ms is already [128 slots, 40 keys]
    nc.sync.dma_start(m_out[:], Ms[:])

    stack.close()
    return m_out


_CACHE = {}


def _get_program():
    if "nc" not in _CACHE:
        import concourse.bacc as bacc
        import concourse.tile as tile
        nc = bacc.Bacc("TRN2", target_bir_lowering=False, debug=False)
        with tile.TileContext(nc) as tc:
            build(nc, tc)
        nc.compile()
        _CACHE["nc"] = nc
    return _CACHE["nc"]


def kernel(**inputs) -> np.ndarray:
    from concourse import bass_utils
    nc = _get_program()
    in_map = _prep(inputs)
    res = bass_utils.run_bass_kernel_spmd(
        nc, [dict(in_map) for _ in range(N_CORES)], core_ids=list(range(N_CORES))
    )
    return res.results[0]["m_out"]
